# revision 20
# baseline (speedup 1.0000x reference)
"""LoRA layer kernel for Trainium2, 8-core data-parallel.

out = x @ W.T + 2.0 * ((x @ B) @ A) = x @ (W.T + 2*(B@A)) = x @ Weff

The LoRA path is folded into the weight on the HOST (B@A is a tiny
rank-16 outer product) so the device kernel is a single dense GEMM:
out[16384, 4096] = x[16384, 4096] @ Weff[4096, 4096].

Sharding: data-parallel over rows, 2048 rows/core, Weff replicated.

Per-core kernel (operands bf16; tolerance 2e-2, bf16 gives ~2e-3):
rows in two resident blocks of 1024; for each block and each 512-wide
output chunk (oc pass), the 8 m-tiles are processed as TWO GROUPS of 4
(4 PSUM banks each) so group A's PSUM evictions complete while group B
computes — the next pass never waits on evictions. W chunks for an oc
pass stay resident across both groups (read once per block). All DMAs
are [128, N]-contiguous host-pre-tiled chunks (descriptor-gen cheap).
Queues: sync=W, scalar=x (+half the evictions), gpsimd=out.
4096 MMs of [128x128x512] @ ~216ns = ~884us PE floor.
"""

import sys

if "/opt/trn_rl_repo" not in sys.path:
    sys.path.insert(0, "/opt/trn_rl_repo")

import os

import numpy as np
import ml_dtypes

import concourse.bass as bass
import concourse.mybir as mybir
import concourse.tile as tile

N_CORES = 8
D = 4096
ROWS_TOTAL = 4 * 4096          # 16384
ROWS_PER_CORE = ROWS_TOTAL // N_CORES  # 2048
P = 128
KT = D // P                    # 32 k-tiles
M_BLOCK = 1024                 # rows per x-resident block
N_BLOCKS = ROWS_PER_CORE // M_BLOCK    # 2
MT_PER_BLOCK = M_BLOCK // P    # 8 m-tiles
GRP = 4                        # m-tiles per PSUM group
OC = 512                       # o-chunk width (one PSUM bank)
N_OC = D // OC                 # 8
KH = KT // 2                   # k-tiles per x half-tile
KQ = 2                         # k-tiles per x DMA chunk
N_KQ = KT // KQ                # 16 chunks
KQW = 2                        # k-tiles per W DMA chunk
N_KQW = KT // KQW              # 16 chunks

F32 = mybir.dt.float32
BF16 = mybir.dt.bfloat16

WARMUP = os.environ.get("K_WARMUP", "1") == "1"


def split_wide_waits(nc, max_waits=1):
    """walrus in this container rejects >1 sync wait per instruction;
    move excess waits onto preceding same-engine NoOps."""
    n_split = 0
    for f in nc.m.functions:
        for bb in f.blocks:
            new_insts = []
            for inst in bb.instructions:
                si = getattr(inst, "sync_info", None)
                if si is not None and si.on_wait and len(si.on_wait) > max_waits:
                    waits = list(si.on_wait)
                    keep = waits[-max_waits:]
                    extra = waits[:-max_waits]
                    for i in range(0, len(extra), max_waits):
                        chunk = extra[i:i + max_waits]
                        nop = mybir.InstNoOp(
                            name=f"{inst.name}_wsplit{i}",
                            sync_info=mybir.SyncInfo(on_wait=chunk, on_update=[]),
                            bass_nofuse=True,
                            engine=inst.engine,
                        )
                        new_insts.append(nop)
                        n_split += 1
                    si.on_wait = keep
                new_insts.append(inst)
            bb.instructions[:] = new_insts
    return n_split


def build_program():
    nc = bass.Bass()
    # xq: pre-tiled x, rows (blk*8+kq)*128.. hold chunk [128, 4*1024]
    xq = nc.declare_dram_parameter("xq", [N_BLOCKS * N_KQ * P, KQ * M_BLOCK], BF16, isOutput=False)
    # wq: pre-tiled Weff, rows (oc*16+kq)*128.. hold chunk [128, 2*512]
    wq = nc.declare_dram_parameter("wq", [N_OC * N_KQW * P, KQW * OC], BF16, isOutput=False)
    cz = nc.declare_dram_parameter("cz", [P, OC], BF16, isOutput=False)
    out = nc.declare_dram_parameter("out", [ROWS_PER_CORE, D], F32, isOutput=True)

    with tile.TileContext(nc) as tc:
        with (
            tc.tile_pool(name="xpool_a", bufs=2) as xpool_a,
            tc.tile_pool(name="xpool_b", bufs=2) as xpool_b,
            tc.tile_pool(name="wpool", bufs=24) as wpool,
            tc.tile_pool(name="opool", bufs=4) as opool,
            tc.tile_pool(name="cpool", bufs=1) as cpool,
            tc.tile_pool(name="ppool", bufs=8, space="PSUM") as ppool,
        ):
            # zeros tile for HAM warmup matmuls (scalar queue: sync stays
            # free so the first W chunk lands as early as possible)
            ztile = cpool.tile([P, OC], BF16, tag="zt")
            nc.scalar.dma_start(ztile[:], cz[:])

            # HAM warmup: dummy matmuls so the PE clock ramps while the
            # first x/W chunks stream in.
            if WARMUP:
                junk = ppool.tile([P, OC], F32, tag="acc", name="junk")
                for i in range(12):
                    nc.tensor.matmul(
                        junk[:],
                        ztile[:, :P],
                        ztile[:],
                        start=(i == 0),
                        stop=(i == 11),
                    )

            xtiles = {}

            def load_x_block(blk):
                xa = xpool_a.tile([P, KH * M_BLOCK], BF16, tag="xa", name=f"xa{blk}")
                xb = xpool_b.tile([P, KH * M_BLOCK], BF16, tag="xb", name=f"xb{blk}")
                xtiles[blk] = (xa, xb)
                for kq in range(N_KQ):
                    t = xa if kq < N_KQ // 2 else xb
                    q0 = (kq % (N_KQ // 2)) * KQ * M_BLOCK
                    rr = (blk * N_KQ + kq) * P
                    # single queue: startup HBM bw splits ~1:1 between the
                    # x (scalar) and W (sync) queues, which matches the
                    # first pass's consumption ratio
                    if blk == 0 and kq == 0:
                        # first chunk halved so the very first k-tile lands
                        # (and the first matmul issues) sooner
                        nc.scalar.dma_start(t[:, q0:q0 + M_BLOCK], xq[rr:rr + P, :M_BLOCK])
                        nc.scalar.dma_start(t[:, q0 + M_BLOCK:q0 + 2 * M_BLOCK], xq[rr:rr + P, M_BLOCK:])
                    else:
                        nc.scalar.dma_start(t[:, q0:q0 + KQ * M_BLOCK], xq[rr:rr + P, :])

            load_x_block(0)

            for blk in range(N_BLOCKS):
                r0 = blk * M_BLOCK
                xa, xb = xtiles[blk]

                def xsl(k, c0, cw):
                    t = xa if k < KH else xb
                    kk = k % KH
                    return t[:, kk * M_BLOCK + c0: kk * M_BLOCK + c0 + cw]

                for oc in range(N_OC):
                    # prefetch x for the next block early (after the first
                    # pass of this block so startup HBM bw isn't stolen)
                    if blk == 0 and oc == 2:
                        load_x_block(1)
                    # W chunks for this oc pass: resident for both groups
                    wtiles = []
                    for kq in range(N_KQW):
                        wtile = wpool.tile([P, KQW * OC], BF16, tag="wt")
                        rr = (oc * N_KQW + kq) * P
                        if blk == 0 and oc == 0 and kq == 0:
                            # halve the first W chunk: first k-tile lands
                            # sooner, first matmul issues earlier
                            nc.sync.dma_start(wtile[:, :OC], wq[rr:rr + P, :OC])
                            nc.sync.dma_start(wtile[:, OC:], wq[rr:rr + P, OC:])
                        else:
                            nc.sync.dma_start(wtile[:], wq[rr:rr + P, :])
                        wtiles.append(wtile)
                    # pass 0 is HBM-startup-bound: one 8-tile group spreads
                    # the x+W first-fetch over the whole 55us pass (a 4-tile
                    # group would need ~460 GB/s > the ~358 GB/s HBM limit).
                    grp_size = MT_PER_BLOCK if (blk == 0 and oc == 0) else GRP
                    for g0 in range(0, MT_PER_BLOCK, grp_size):
                        psums = []
                        for mi in range(grp_size):
                            psums.append(ppool.tile(
                                [P, OC], F32, tag="acc",
                                name=f"ps_{blk}_{oc}_{g0}_{mi}"))
                        for kq in range(N_KQW):
                            for kk in range(KQW):
                                k = KQW * kq + kk
                                for mi in range(grp_size):
                                    mt = g0 + mi
                                    nc.tensor.matmul(
                                        psums[mi][:],
                                        xsl(k, mt * P, P),
                                        wtiles[kq][:, kk * OC:(kk + 1) * OC],
                                        start=(k == 0),
                                        stop=(k == KT - 1),
                                    )
                        last_pass = (blk == N_BLOCKS - 1 and oc == N_OC - 1)
                        for mi in range(grp_size):
                            mt = g0 + mi
                            ot = opool.tile([P, OC], F32, tag="ot")
                            if mi % 2 == 0:
                                nc.vector.tensor_copy(ot[:], psums[mi][:])
                            else:
                                nc.scalar.copy(ot[:], psums[mi][:])
                            # out via SWDGE: sync HWDGE stays dedicated to W.
                            # Final pass: W queue is done, so drain the last
                            # tiles via the faster HWDGE queues instead.
                            if last_pass:
                                deng = nc.sync if mi % 2 == 0 else nc.scalar
                            else:
                                deng = nc.gpsimd
                            deng.dma_start(
                                out[r0 + mt * P:r0 + (mt + 1) * P,
                                    oc * OC:(oc + 1) * OC],
                                ot[:],
                            )

    split_wide_waits(nc)
    return nc


_NC_CACHE = [None]


def _pretile_w(weff_bf):
    # [4096, 4096] -> [8 oc, 16 kq, 128 p, 2 kk, 512 c] -> [16384, 1024]
    w = weff_bf.reshape(N_KQW, KQW, P, N_OC, OC)        # kq, kk, p, oc, c
    w = w.transpose(3, 0, 2, 1, 4)                      # oc, kq, p, kk, c
    return np.ascontiguousarray(w).reshape(N_OC * N_KQW * P, KQW * OC)


def _pretile_x(xt_c):
    # xt_c: [4096, 2048] (k-major, rows for this core transposed)
    # -> [2 blk, 8 kq, 128 p, 4 q, 1024 m] -> [2048, 4096]
    xv = xt_c.reshape(N_KQ, KQ, P, N_BLOCKS, M_BLOCK)   # kq, q, p, blk, m
    xv = xv.transpose(3, 0, 2, 1, 4)                    # blk, kq, p, q, m
    return np.ascontiguousarray(xv).reshape(N_BLOCKS * N_KQ * P, KQ * M_BLOCK)


def kernel(x, weight, lora_A, lora_B):
    from concourse.bass_utils import run_bass_kernel_spmd

    x = np.asarray(x, dtype=np.float32)
    weight = np.asarray(weight, dtype=np.float32)
    lora_A = np.asarray(lora_A, dtype=np.float32)
    lora_B = np.asarray(lora_B, dtype=np.float32)

    # fold LoRA into the weight: out = x @ (W.T + 2*(B@A))
    weff = weight.T + 2.0 * (lora_B @ lora_A)
    wq = _pretile_w(weff.astype(ml_dtypes.bfloat16))

    x2 = x.reshape(ROWS_TOTAL, D).astype(ml_dtypes.bfloat16)
    cz = np.zeros((P, OC), dtype=ml_dtypes.bfloat16)

    in_maps = []
    for c in range(N_CORES):
        xt_c = np.ascontiguousarray(
            x2[c * ROWS_PER_CORE:(c + 1) * ROWS_PER_CORE].T
        )
        in_maps.append({"xq": _pretile_x(xt_c), "wq": wq, "cz": cz})

    if _NC_CACHE[0] is None:
        _NC_CACHE[0] = build_program()
    nc = _NC_CACHE[0]

    res = run_bass_kernel_spmd(nc, in_maps, list(range(N_CORES)))
    out = np.concatenate(
        [res.results[c]["out"] for c in range(N_CORES)], axis=0
    )
    return out.reshape(x.shape)


# revision 24
# speedup vs baseline: 1.1978x; 1.1978x over previous
"""LoRA layer kernel for Trainium2, 8-core data-parallel.

out = x @ W.T + 2.0 * ((x @ B) @ A) = x @ (W.T + 2*(B@A)) = x @ Weff

The LoRA path is folded into the weight on the HOST (B@A is a tiny
rank-16 outer product) so the device kernel is a single dense GEMM:
out[16384, 4096] = x[16384, 4096] @ Weff[4096, 4096].

Sharding: data-parallel over rows, 2048 rows/core, Weff replicated.

Per-core kernel (operands bf16; tolerance 2e-2, bf16 gives ~2e-3):
rows in two resident blocks of 1024; for each block and each 512-wide
output chunk (oc pass), the 8 m-tiles are processed as TWO GROUPS of 4
(4 PSUM banks each) so group A's PSUM evictions complete while group B
computes — the next pass never waits on evictions. W chunks for an oc
pass stay resident across both groups (read once per block). All DMAs
are [128, N]-contiguous host-pre-tiled chunks (descriptor-gen cheap).
Queues: sync=W, scalar=x (+half the evictions), gpsimd=out.
4096 MMs of [128x128x512] @ ~216ns = ~884us PE floor.
"""

import sys

if "/opt/trn_rl_repo" not in sys.path:
    sys.path.insert(0, "/opt/trn_rl_repo")

import os

import numpy as np
import ml_dtypes

import concourse.bass as bass
import concourse.mybir as mybir
import concourse.tile as tile

N_CORES = 8
D = 4096
ROWS_TOTAL = 4 * 4096          # 16384
ROWS_PER_CORE = ROWS_TOTAL // N_CORES  # 2048
P = 128
KT = D // P                    # 32 k-tiles
M_BLOCK = 1024                 # rows per x-resident block
N_BLOCKS = ROWS_PER_CORE // M_BLOCK    # 2
MT_PER_BLOCK = M_BLOCK // P    # 8 m-tiles
GRP = 4                        # m-tiles per PSUM group
OC = 512                       # o-chunk width (one PSUM bank)
N_OC = D // OC                 # 8
KH = KT // 2                   # k-tiles per x half-tile
KQ = 2                         # k-tiles per x DMA chunk
N_KQ = KT // KQ                # 16 chunks
KQW = 2                        # k-tiles per W DMA chunk
N_KQW = KT // KQW              # 16 chunks

F32 = mybir.dt.float32
BF16 = mybir.dt.bfloat16

WARMUP = os.environ.get("K_WARMUP", "1") == "1"


def split_wide_waits(nc, max_waits=1):
    """walrus in this container rejects >1 sync wait per instruction;
    move excess waits onto preceding same-engine NoOps."""
    n_split = 0
    for f in nc.m.functions:
        for bb in f.blocks:
            new_insts = []
            for inst in bb.instructions:
                si = getattr(inst, "sync_info", None)
                if si is not None and si.on_wait and len(si.on_wait) > max_waits:
                    waits = list(si.on_wait)
                    keep = waits[-max_waits:]
                    extra = waits[:-max_waits]
                    for i in range(0, len(extra), max_waits):
                        chunk = extra[i:i + max_waits]
                        nop = mybir.InstNoOp(
                            name=f"{inst.name}_wsplit{i}",
                            sync_info=mybir.SyncInfo(on_wait=chunk, on_update=[]),
                            bass_nofuse=True,
                            engine=inst.engine,
                        )
                        new_insts.append(nop)
                        n_split += 1
                    si.on_wait = keep
                new_insts.append(inst)
            bb.instructions[:] = new_insts
    return n_split


def build_program():
    nc = bass.Bass()
    # xq: pre-tiled x, rows (blk*8+kq)*128.. hold chunk [128, 4*1024]
    xq = nc.declare_dram_parameter("xq", [N_BLOCKS * N_KQ * P, KQ * M_BLOCK], BF16, isOutput=False)
    # wq: pre-tiled Weff, rows (oc*16+kq)*128.. hold chunk [128, 2*512]
    wq = nc.declare_dram_parameter("wq", [N_OC * N_KQW * P, KQW * OC], BF16, isOutput=False)
    cz = nc.declare_dram_parameter("cz", [P, OC], BF16, isOutput=False)
    out = nc.declare_dram_parameter("out", [ROWS_PER_CORE, D], F32, isOutput=True)

    with tile.TileContext(nc) as tc:
        with (
            tc.tile_pool(name="xpool_a", bufs=2) as xpool_a,
            tc.tile_pool(name="xpool_b", bufs=2) as xpool_b,
            tc.tile_pool(name="wpool", bufs=24) as wpool,
            tc.tile_pool(name="opool", bufs=4) as opool,
            tc.tile_pool(name="cpool", bufs=1) as cpool,
            tc.tile_pool(name="ppool", bufs=8, space="PSUM") as ppool,
        ):
            # zeros tile for HAM warmup matmuls (scalar queue: sync stays
            # free so the first W chunk lands as early as possible)
            ztile = cpool.tile([P, OC], BF16, tag="zt")
            nc.scalar.dma_start(ztile[:], cz[:])

            # HAM warmup: dummy matmuls so the PE clock ramps while the
            # first x/W chunks stream in.
            if WARMUP:
                junk = ppool.tile([P, OC], F32, tag="acc", name="junk")
                for i in range(12):
                    nc.tensor.matmul(
                        junk[:],
                        ztile[:, :P],
                        ztile[:],
                        start=(i == 0),
                        stop=(i == 11),
                    )

            xtiles = {}

            def load_x_block(blk):
                xa = xpool_a.tile([P, KH * M_BLOCK], BF16, tag="xa", name=f"xa{blk}")
                xb = xpool_b.tile([P, KH * M_BLOCK], BF16, tag="xb", name=f"xb{blk}")
                xtiles[blk] = (xa, xb)
                for kq in range(N_KQ):
                    t = xa if kq < N_KQ // 2 else xb
                    q0 = (kq % (N_KQ // 2)) * KQ * M_BLOCK
                    rr = (blk * N_KQ + kq) * P
                    # single queue: startup HBM bw splits ~1:1 between the
                    # x (scalar) and W (sync) queues, which matches the
                    # first pass's consumption ratio
                    if blk == 0 and kq == 0:
                        # first chunk halved AND routed via sync (ahead of
                        # the W chunks in that FIFO) so the very first
                        # k-tile lands, and the first matmul issues, sooner
                        nc.sync.dma_start(t[:, q0:q0 + M_BLOCK], xq[rr:rr + P, :M_BLOCK])
                        nc.sync.dma_start(t[:, q0 + M_BLOCK:q0 + 2 * M_BLOCK], xq[rr:rr + P, M_BLOCK:])
                    else:
                        nc.scalar.dma_start(t[:, q0:q0 + KQ * M_BLOCK], xq[rr:rr + P, :])

            load_x_block(0)

            for blk in range(N_BLOCKS):
                r0 = blk * M_BLOCK
                xa, xb = xtiles[blk]

                def xsl(k, c0, cw):
                    t = xa if k < KH else xb
                    kk = k % KH
                    return t[:, kk * M_BLOCK + c0: kk * M_BLOCK + c0 + cw]

                for oc in range(N_OC):
                    # prefetch x for the next block early (after the first
                    # pass of this block so startup HBM bw isn't stolen)
                    if blk == 0 and oc == 2:
                        load_x_block(1)
                    # W chunks for this oc pass: resident for both groups
                    wtiles = []
                    for kq in range(N_KQW):
                        wtile = wpool.tile([P, KQW * OC], BF16, tag="wt")
                        rr = (oc * N_KQW + kq) * P
                        if blk == 0 and oc == 0 and kq == 0:
                            # halve the first W chunk: first k-tile lands
                            # sooner, first matmul issues earlier
                            nc.sync.dma_start(wtile[:, :OC], wq[rr:rr + P, :OC])
                            nc.sync.dma_start(wtile[:, OC:], wq[rr:rr + P, OC:])
                        else:
                            nc.sync.dma_start(wtile[:], wq[rr:rr + P, :])
                        wtiles.append(wtile)
                    # pass 0 is HBM-startup-bound: a big 7-tile group spreads
                    # the x+W first-fetch over most of the pass (a 4-tile
                    # group would need ~460 GB/s > the ~358 GB/s HBM limit);
                    # the trailing 1-tile group then hides the 7 evictions so
                    # pass 1 never waits on them. The final pass uses the
                    # same (7,1) shape so the tail drains a single tile.
                    first_pass = (blk == 0 and oc == 0)
                    last_pass = (blk == N_BLOCKS - 1 and oc == N_OC - 1)
                    grp_sizes = [7, 1] if (first_pass or last_pass) else [GRP, GRP]
                    g0 = 0
                    for grp_size in grp_sizes:
                        psums = []
                        for mi in range(grp_size):
                            psums.append(ppool.tile(
                                [P, OC], F32, tag="acc",
                                name=f"ps_{blk}_{oc}_{g0}_{mi}"))
                        for kq in range(N_KQW):
                            for kk in range(KQW):
                                k = KQW * kq + kk
                                for mi in range(grp_size):
                                    mt = g0 + mi
                                    nc.tensor.matmul(
                                        psums[mi][:],
                                        xsl(k, mt * P, P),
                                        wtiles[kq][:, kk * OC:(kk + 1) * OC],
                                        start=(k == 0),
                                        stop=(k == KT - 1),
                                    )
                        for mi in range(grp_size):
                            mt = g0 + mi
                            ot = opool.tile([P, OC], F32, tag="ot")
                            if mi % 2 == 0:
                                nc.vector.tensor_copy(ot[:], psums[mi][:])
                            else:
                                nc.scalar.copy(ot[:], psums[mi][:])
                            # out via SWDGE: sync HWDGE stays dedicated to W.
                            # Final pass: W queue is done, so drain the last
                            # tiles via the faster HWDGE queues instead.
                            if last_pass:
                                deng = nc.sync if mi % 2 == 0 else nc.scalar
                            else:
                                deng = nc.gpsimd
                            deng.dma_start(
                                out[r0 + mt * P:r0 + (mt + 1) * P,
                                    oc * OC:(oc + 1) * OC],
                                ot[:],
                            )
                        g0 += grp_size

    split_wide_waits(nc)
    return nc


_NC_CACHE = [None]


def _pretile_w(weff_bf):
    # [4096, 4096] -> [8 oc, 16 kq, 128 p, 2 kk, 512 c] -> [16384, 1024]
    w = weff_bf.reshape(N_KQW, KQW, P, N_OC, OC)        # kq, kk, p, oc, c
    w = w.transpose(3, 0, 2, 1, 4)                      # oc, kq, p, kk, c
    return np.ascontiguousarray(w).reshape(N_OC * N_KQW * P, KQW * OC)


def _pretile_x(xt_c):
    # xt_c: [4096, 2048] (k-major, rows for this core transposed)
    # -> [2 blk, 8 kq, 128 p, 4 q, 1024 m] -> [2048, 4096]
    xv = xt_c.reshape(N_KQ, KQ, P, N_BLOCKS, M_BLOCK)   # kq, q, p, blk, m
    xv = xv.transpose(3, 0, 2, 1, 4)                    # blk, kq, p, q, m
    return np.ascontiguousarray(xv).reshape(N_BLOCKS * N_KQ * P, KQ * M_BLOCK)


def kernel(x, weight, lora_A, lora_B):
    from concourse.bass_utils import run_bass_kernel_spmd

    x = np.asarray(x, dtype=np.float32)
    weight = np.asarray(weight, dtype=np.float32)
    lora_A = np.asarray(lora_A, dtype=np.float32)
    lora_B = np.asarray(lora_B, dtype=np.float32)

    # fold LoRA into the weight: out = x @ (W.T + 2*(B@A))
    weff = weight.T + 2.0 * (lora_B @ lora_A)
    wq = _pretile_w(weff.astype(ml_dtypes.bfloat16))

    x2 = x.reshape(ROWS_TOTAL, D).astype(ml_dtypes.bfloat16)
    cz = np.zeros((P, OC), dtype=ml_dtypes.bfloat16)

    in_maps = []
    for c in range(N_CORES):
        xt_c = np.ascontiguousarray(
            x2[c * ROWS_PER_CORE:(c + 1) * ROWS_PER_CORE].T
        )
        in_maps.append({"xq": _pretile_x(xt_c), "wq": wq, "cz": cz})

    if _NC_CACHE[0] is None:
        _NC_CACHE[0] = build_program()
    nc = _NC_CACHE[0]

    res = run_bass_kernel_spmd(nc, in_maps, list(range(N_CORES)))
    out = np.concatenate(
        [res.results[c]["out"] for c in range(N_CORES)], axis=0
    )
    return out.reshape(x.shape)


# revision 28
# speedup vs baseline: 1.2003x; 1.0021x over previous
"""LoRA layer kernel for Trainium2, 8-core data-parallel.

out = x @ W.T + 2.0 * ((x @ B) @ A) = x @ (W.T + 2*(B@A)) = x @ Weff

The LoRA path is folded into the weight on the HOST (B@A is a tiny
rank-16 outer product) so the device kernel is a single dense GEMM:
out[16384, 4096] = x[16384, 4096] @ Weff[4096, 4096].

Sharding: data-parallel over rows, 2048 rows/core, Weff replicated.

Per-core kernel (operands bf16; tolerance 2e-2, bf16 gives ~2e-3):
rows in two resident blocks of 1024; for each block and each 512-wide
output chunk (oc pass), the 8 m-tiles are processed as TWO GROUPS of 4
(4 PSUM banks each) so group A's PSUM evictions complete while group B
computes — the next pass never waits on evictions. W chunks for an oc
pass stay resident across both groups (read once per block). All DMAs
are [128, N]-contiguous host-pre-tiled chunks (descriptor-gen cheap).
Queues: sync=W, scalar=x (+half the evictions), gpsimd=out.
4096 MMs of [128x128x512] @ ~216ns = ~884us PE floor.
"""

import sys

if "/opt/trn_rl_repo" not in sys.path:
    sys.path.insert(0, "/opt/trn_rl_repo")

import os

import numpy as np
import ml_dtypes

import concourse.bass as bass
import concourse.mybir as mybir
import concourse.tile as tile

N_CORES = 8
D = 4096
ROWS_TOTAL = 4 * 4096          # 16384
ROWS_PER_CORE = ROWS_TOTAL // N_CORES  # 2048
P = 128
KT = D // P                    # 32 k-tiles
M_BLOCK = 1024                 # rows per x-resident block
N_BLOCKS = ROWS_PER_CORE // M_BLOCK    # 2
MT_PER_BLOCK = M_BLOCK // P    # 8 m-tiles
GRP = 4                        # m-tiles per PSUM group
OC = 512                       # o-chunk width (one PSUM bank)
N_OC = D // OC                 # 8
KH = KT // 2                   # k-tiles per x half-tile
KQ = 2                         # k-tiles per x DMA chunk
N_KQ = KT // KQ                # 16 chunks
KQW = 2                        # k-tiles per W DMA chunk
N_KQW = KT // KQW              # 16 chunks

F32 = mybir.dt.float32
BF16 = mybir.dt.bfloat16

WARMUP = os.environ.get("K_WARMUP", "1") == "1"


def split_wide_waits(nc, max_waits=1):
    """walrus in this container rejects >1 sync wait per instruction;
    move excess waits onto preceding same-engine NoOps."""
    n_split = 0
    for f in nc.m.functions:
        for bb in f.blocks:
            new_insts = []
            for inst in bb.instructions:
                si = getattr(inst, "sync_info", None)
                if si is not None and si.on_wait and len(si.on_wait) > max_waits:
                    waits = list(si.on_wait)
                    keep = waits[-max_waits:]
                    extra = waits[:-max_waits]
                    for i in range(0, len(extra), max_waits):
                        chunk = extra[i:i + max_waits]
                        nop = mybir.InstNoOp(
                            name=f"{inst.name}_wsplit{i}",
                            sync_info=mybir.SyncInfo(on_wait=chunk, on_update=[]),
                            bass_nofuse=True,
                            engine=inst.engine,
                        )
                        new_insts.append(nop)
                        n_split += 1
                    si.on_wait = keep
                new_insts.append(inst)
            bb.instructions[:] = new_insts
    return n_split


def build_program():
    nc = bass.Bass()
    # xq: pre-tiled x, rows (blk*8+kq)*128.. hold chunk [128, 4*1024]
    xq = nc.declare_dram_parameter("xq", [N_BLOCKS * N_KQ * P, KQ * M_BLOCK], BF16, isOutput=False)
    # wq: pre-tiled Weff, rows (oc*16+kq)*128.. hold chunk [128, 2*512]
    wq = nc.declare_dram_parameter("wq", [N_OC * N_KQW * P, KQW * OC], BF16, isOutput=False)
    cz = nc.declare_dram_parameter("cz", [P, OC], BF16, isOutput=False)
    out = nc.declare_dram_parameter("out", [ROWS_PER_CORE, D], F32, isOutput=True)

    with tile.TileContext(nc) as tc:
        with (
            tc.tile_pool(name="xpool_a", bufs=2) as xpool_a,
            tc.tile_pool(name="xpool_b", bufs=2) as xpool_b,
            tc.tile_pool(name="wpool", bufs=24) as wpool,
            tc.tile_pool(name="opool", bufs=4) as opool,
            tc.tile_pool(name="cpool", bufs=1) as cpool,
            tc.tile_pool(name="ppool", bufs=8, space="PSUM") as ppool,
        ):
            # zeros tile for HAM warmup matmuls (scalar queue: sync stays
            # free so the first W chunk lands as early as possible)
            ztile = cpool.tile([P, OC], BF16, tag="zt")
            nc.scalar.dma_start(ztile[:], cz[:])

            # HAM warmup: dummy matmuls so the PE clock ramps while the
            # first x/W chunks stream in.
            if WARMUP:
                junk = ppool.tile([P, OC], F32, tag="acc", name="junk")
                for i in range(12):
                    nc.tensor.matmul(
                        junk[:],
                        ztile[:, :P],
                        ztile[:],
                        start=(i == 0),
                        stop=(i == 11),
                    )

            xtiles = {}

            def load_x_block(blk, pre=None):
                if pre is not None:
                    xa, xb = pre
                else:
                    xa = xpool_a.tile([P, KH * M_BLOCK], BF16, tag="xa", name=f"xa{blk}")
                    xb = xpool_b.tile([P, KH * M_BLOCK], BF16, tag="xb", name=f"xb{blk}")
                xtiles[blk] = (xa, xb)
                for kq in range(N_KQ):
                    if blk == 0 and kq == 0:
                        continue  # loaded by the startup ladder below
                    t = xa if kq < N_KQ // 2 else xb
                    q0 = (kq % (N_KQ // 2)) * KQ * M_BLOCK
                    rr = (blk * N_KQ + kq) * P
                    # single queue: startup HBM bw splits ~1:1 between the
                    # x (scalar) and W (sync) queues, which matches the
                    # first pass's consumption ratio
                    nc.scalar.dma_start(t[:, q0:q0 + KQ * M_BLOCK], xq[rr:rr + P, :])

            # startup ladder on sync: the first matmul needs x k-tile 0 AND
            # W k-tile 0 — interleave half-chunks so that pair completes
            # first, then k-tile 1, before the bulk streams begin.
            xa0 = xpool_a.tile([P, KH * M_BLOCK], BF16, tag="xa", name="xa0")
            xb0 = xpool_b.tile([P, KH * M_BLOCK], BF16, tag="xb", name="xb0")
            w00 = wpool.tile([P, KQW * OC], BF16, tag="wt", name="w00")
            nc.sync.dma_start(xa0[:, :M_BLOCK], xq[:P, :M_BLOCK])
            nc.sync.dma_start(w00[:, :OC], wq[:P, :OC])
            nc.sync.dma_start(xa0[:, M_BLOCK:2 * M_BLOCK], xq[:P, M_BLOCK:])
            nc.sync.dma_start(w00[:, OC:], wq[:P, OC:])

            load_x_block(0, pre=(xa0, xb0))

            for blk in range(N_BLOCKS):
                r0 = blk * M_BLOCK
                xa, xb = xtiles[blk]

                def xsl(k, c0, cw):
                    t = xa if k < KH else xb
                    kk = k % KH
                    return t[:, kk * M_BLOCK + c0: kk * M_BLOCK + c0 + cw]

                for oc in range(N_OC):
                    # prefetch x for the next block early (after the first
                    # pass of this block so startup HBM bw isn't stolen)
                    if blk == 0 and oc == 2:
                        load_x_block(1)
                    # W chunks for this oc pass: resident for both groups
                    wtiles = []
                    for kq in range(N_KQW):
                        if blk == 0 and oc == 0 and kq == 0:
                            wtiles.append(w00)  # loaded by the startup ladder
                            continue
                        wtile = wpool.tile([P, KQW * OC], BF16, tag="wt")
                        rr = (oc * N_KQW + kq) * P
                        nc.sync.dma_start(wtile[:], wq[rr:rr + P, :])
                        wtiles.append(wtile)
                    # pass 0 is HBM-startup-bound: a big 7-tile group spreads
                    # the x+W first-fetch over most of the pass (a 4-tile
                    # group would need ~460 GB/s > the ~358 GB/s HBM limit);
                    # the trailing 1-tile group then hides the 7 evictions so
                    # pass 1 never waits on them. The final pass uses the
                    # same (7,1) shape so the tail drains a single tile.
                    first_pass = (blk == 0 and oc == 0)
                    last_pass = (blk == N_BLOCKS - 1 and oc == N_OC - 1)
                    grp_sizes = [7, 1] if (first_pass or last_pass) else [GRP, GRP]
                    g0 = 0
                    for grp_size in grp_sizes:
                        psums = []
                        for mi in range(grp_size):
                            psums.append(ppool.tile(
                                [P, OC], F32, tag="acc",
                                name=f"ps_{blk}_{oc}_{g0}_{mi}"))
                        for kq in range(N_KQW):
                            for kk in range(KQW):
                                k = KQW * kq + kk
                                for mi in range(grp_size):
                                    mt = g0 + mi
                                    nc.tensor.matmul(
                                        psums[mi][:],
                                        xsl(k, mt * P, P),
                                        wtiles[kq][:, kk * OC:(kk + 1) * OC],
                                        start=(k == 0),
                                        stop=(k == KT - 1),
                                    )
                        for mi in range(grp_size):
                            mt = g0 + mi
                            ot = opool.tile([P, OC], F32, tag="ot")
                            if mi % 2 == 0:
                                nc.vector.tensor_copy(ot[:], psums[mi][:])
                            else:
                                nc.scalar.copy(ot[:], psums[mi][:])
                            # out via SWDGE: sync HWDGE stays dedicated to W.
                            # Final pass: W queue is done, so drain the last
                            # tiles via the faster HWDGE queues instead.
                            if last_pass:
                                deng = nc.sync if mi % 2 == 0 else nc.scalar
                            else:
                                deng = nc.gpsimd
                            deng.dma_start(
                                out[r0 + mt * P:r0 + (mt + 1) * P,
                                    oc * OC:(oc + 1) * OC],
                                ot[:],
                            )
                        g0 += grp_size

    split_wide_waits(nc)
    return nc


_NC_CACHE = [None]


def _pretile_w(weff_bf):
    # [4096, 4096] -> [8 oc, 16 kq, 128 p, 2 kk, 512 c] -> [16384, 1024]
    w = weff_bf.reshape(N_KQW, KQW, P, N_OC, OC)        # kq, kk, p, oc, c
    w = w.transpose(3, 0, 2, 1, 4)                      # oc, kq, p, kk, c
    return np.ascontiguousarray(w).reshape(N_OC * N_KQW * P, KQW * OC)


def _pretile_x(xt_c):
    # xt_c: [4096, 2048] (k-major, rows for this core transposed)
    # -> [2 blk, 8 kq, 128 p, 4 q, 1024 m] -> [2048, 4096]
    xv = xt_c.reshape(N_KQ, KQ, P, N_BLOCKS, M_BLOCK)   # kq, q, p, blk, m
    xv = xv.transpose(3, 0, 2, 1, 4)                    # blk, kq, p, q, m
    return np.ascontiguousarray(xv).reshape(N_BLOCKS * N_KQ * P, KQ * M_BLOCK)


def kernel(x, weight, lora_A, lora_B):
    from concourse.bass_utils import run_bass_kernel_spmd

    x = np.asarray(x, dtype=np.float32)
    weight = np.asarray(weight, dtype=np.float32)
    lora_A = np.asarray(lora_A, dtype=np.float32)
    lora_B = np.asarray(lora_B, dtype=np.float32)

    # fold LoRA into the weight: out = x @ (W.T + 2*(B@A))
    weff = weight.T + 2.0 * (lora_B @ lora_A)
    wq = _pretile_w(weff.astype(ml_dtypes.bfloat16))

    x2 = x.reshape(ROWS_TOTAL, D).astype(ml_dtypes.bfloat16)
    cz = np.zeros((P, OC), dtype=ml_dtypes.bfloat16)

    in_maps = []
    for c in range(N_CORES):
        xt_c = np.ascontiguousarray(
            x2[c * ROWS_PER_CORE:(c + 1) * ROWS_PER_CORE].T
        )
        in_maps.append({"xq": _pretile_x(xt_c), "wq": wq, "cz": cz})

    if _NC_CACHE[0] is None:
        _NC_CACHE[0] = build_program()
    nc = _NC_CACHE[0]

    res = run_bass_kernel_spmd(nc, in_maps, list(range(N_CORES)))
    out = np.concatenate(
        [res.results[c]["out"] for c in range(N_CORES)], axis=0
    )
    return out.reshape(x.shape)
